# revision 3
# baseline (speedup 1.0000x reference)
"""Talking-heads attention kernel for Trainium2, 8 NeuronCores.

Problem: B=4, N=2048, DIM=512, H=8, DH=64 talking-heads attention
(qkv proj -> per-head scores -> th1 head-mix -> softmax -> th2 head-mix
 -> attn @ v -> out proj).

Sharding: data-parallel over (batch, query-half): core c handles batch c//2,
query rows [1024*(c%2), 1024*(c%2)+1024).  Communication-free.

Device pipeline per core (all matmuls bf16 with fp32 PSUM accumulation):
  1. x -> bf16 -> xbar-transpose -> x^T; QKV projection on TensorE producing
     Q^T/K^T (head-transposed layout) and V (n-major).
  2. Per 128-query tile: per-head scores S_h = Q_h K_h^T (scale folded into
     w_q on host).
  3. Pack scores into head-interleaved tiles [(n16,h), m] via SBUF->SBUF DMA,
     then both talking-heads mixes run as full-width 128x128 block-diagonal
     matmuls on TensorE.
  4. exp on ScalarE with fused per-row accumulation (softmax denominator Z
     comes for free); the softmax division is folded into the second mix's
     stationary weights (th2[g,h]/Z row scaling), so no elementwise divide
     pass ever touches the [n,m] matrix.
  5. mix2 output is xbar-DMA-transposed to key-major A^T, attn@V runs with
     V as the stationary operand, followed by the output projection.
"""

import sys

sys.path.insert(0, "/opt/trn_rl_repo")

import numpy as np
import ml_dtypes

import concourse.bass as bass
from concourse import bacc
import concourse.mybir as mybir
import concourse.tile as tile
from concourse.bass_utils import run_bass_kernel_spmd

BF16 = mybir.dt.bfloat16
F32 = mybir.dt.float32
AF = mybir.ActivationFunctionType

B, N, DIM = 4, 2048, 512
H, DH = 8, 64
NCORES = 8
NLOC = N // 2          # query rows per core
NT = NLOC // 128       # 8 query tiles per core
MT = N // 128          # 16 key chunks
NG = 16                # n16 group size in interleaved tiles


def build_nc():
    nc = bacc.Bacc()

    x = nc.declare_dram_parameter("x", [N, DIM], F32, isOutput=False)
    xq = nc.declare_dram_parameter("xq", [NLOC, DIM], F32, isOutput=False)
    wq = nc.declare_dram_parameter("wq", [DIM, DIM], BF16, isOutput=False)
    wk = nc.declare_dram_parameter("wk", [DIM, DIM], BF16, isOutput=False)
    wv = nc.declare_dram_parameter("wv", [DIM, DIM], BF16, isOutput=False)
    wo = nc.declare_dram_parameter("wo", [DIM, DIM], BF16, isOutput=False)
    t1t = nc.declare_dram_parameter("t1t", [128, 128], BF16, isOutput=False)
    t2t = nc.declare_dram_parameter("t2t", [128, 128], BF16, isOutput=False)
    # rows: bq (pre-scaled), bk, bv, bo
    bia = nc.declare_dram_parameter("bia", [1, 4 * DIM], BF16, isOutput=False)
    y = nc.declare_dram_parameter("y", [NLOC, DIM], F32, isOutput=True)

    with tile.TileContext(nc) as tc:
        with (
            tc.tile_pool(name="pw", bufs=1) as pw,      # persistent across phases
            tc.tile_pool(name="psA", bufs=2, space="PSUM") as psA,  # [128,1024]
            tc.tile_pool(name="psB", bufs=2, space="PSUM") as psB,  # [128,512]
            tc.tile_pool(name="psV", bufs=2, space="PSUM") as psV,  # [64,128]
        ):
            # persistent small tensors
            wo_sb = [pw.tile([128, DIM], BF16, name=f"wo{i}", tag=f"wo{i}") for i in range(4)]
            for i in range(4):
                nc.sync.dma_start(wo_sb[i][:], wo[128 * i:128 * (i + 1), :])
            t1_sb = pw.tile([128, 128], BF16, name="t1", tag="t1")
            t2_sb = pw.tile([128, 128], BF16, name="t2", tag="t2")
            nc.sync.dma_start(t1_sb[:], t1t[:])
            nc.sync.dma_start(t2_sb[:], t2t[:])
            bia_sb = pw.tile([1, 4 * DIM], BF16, name="bia", tag="bia")
            nc.sync.dma_start(bia_sb[:], bia[:])
            ones_sb = pw.tile([1, DIM], BF16, name="ones", tag="ones")
            nc.any.memset(ones_sb[:], 1.0)

            # persistent activations: Q^T/K^T (head-transposed), V (n-major)
            qt_sb = [pw.tile([128, NLOC], BF16, name=f"qt{i}", tag=f"qt{i}") for i in range(4)]
            kt_sb = [pw.tile([128, N], BF16, name=f"kt{i}", tag=f"kt{i}") for i in range(4)]
            v_sb = pw.tile([128, MT, DIM], BF16, name="v", tag="v")

            def evict(dst, src, use_act):
                if use_act:
                    nc.scalar.copy(dst, src)
                else:
                    nc.vector.tensor_copy(dst, src)

            # ================= phase A: x^T + QKV projection =================
            with tc.tile_pool(name="pxt", bufs=1) as pxt:
                wq_sb = [pxt.tile([128, DIM], BF16, name=f"wq{i}", tag=f"wq{i}") for i in range(4)]
                wk_sb = [pxt.tile([128, DIM], BF16, name=f"wk{i}", tag=f"wk{i}") for i in range(4)]
                wv_sb = [pxt.tile([128, DIM], BF16, name=f"wv{i}", tag=f"wv{i}") for i in range(4)]
                for i in range(4):
                    nc.sync.dma_start(wq_sb[i][:], wq[128 * i:128 * (i + 1), :])
                    nc.sync.dma_start(wk_sb[i][:], wk[128 * i:128 * (i + 1), :])
                    nc.sync.dma_start(wv_sb[i][:], wv[128 * i:128 * (i + 1), :])

                # x -> x^T (bf16): cast-DMA then xbar transpose
                # xt_sb[p, t, j, f]: dim = j*128+p, key row m = t*128+f
                xt_sb = pxt.tile([128, MT, 4, 128], BF16, name="xt", tag="xt")
                for t in range(MT):
                    xb = pxt.tile([128, DIM], BF16, name="xb", tag="xb", bufs=4)
                    nc.gpsimd.dma_start(xb[:], x[128 * t:128 * (t + 1), :])
                    nc.sync.dma_start_transpose(xt_sb[:, t, :, :], xb[:])
                xqt_sb = pxt.tile([128, NT, 4, 128], BF16, name="xqt", tag="xqt")
                for t in range(NT):
                    xb = pxt.tile([128, DIM], BF16, name="xb", tag="xb", bufs=4)
                    nc.gpsimd.dma_start(xb[:], xq[128 * t:128 * (t + 1), :])
                    nc.sync.dma_start_transpose(xqt_sb[:, t, :, :], xb[:])

                ei = 0
                for rc in range(4):            # (h,d) row chunk
                    for nch in range(NLOC // 512):   # Q^T over own query half
                        ps = psB.tile([128, 512], F32, name="psq", tag="psq")
                        for j in range(4):
                            rhs = xqt_sb[:, 4 * nch:4 * (nch + 1), j, :]
                            nc.tensor.matmul(ps[:], wq_sb[j][:, 128 * rc:128 * (rc + 1)],
                                             rhs, start=(j == 0), stop=False)
                        nc.tensor.matmul(ps[:], bia_sb[0:1, 0 * DIM + 128 * rc:0 * DIM + 128 * (rc + 1)],
                                         ones_sb[:, 0:512], start=False, stop=True)
                        evict(qt_sb[rc][:, 512 * nch:512 * (nch + 1)], ps[:], ei % 2 == 0)
                        ei += 1
                    for mch in range(N // 512):      # K^T over all keys
                        ps = psB.tile([128, 512], F32, name="psq", tag="psq")
                        for j in range(4):
                            rhs = xt_sb[:, 4 * mch:4 * (mch + 1), j, :]
                            nc.tensor.matmul(ps[:], wk_sb[j][:, 128 * rc:128 * (rc + 1)],
                                             rhs, start=(j == 0), stop=False)
                        nc.tensor.matmul(ps[:], bia_sb[0:1, 1 * DIM + 128 * rc:1 * DIM + 128 * (rc + 1)],
                                         ones_sb[:, 0:512], start=False, stop=True)
                        evict(kt_sb[rc][:, 512 * mch:512 * (mch + 1)], ps[:], ei % 2 == 0)
                        ei += 1
                for mt in range(MT):           # V n-major
                    ps = psB.tile([128, 512], F32, name="psq", tag="psq")
                    for j in range(4):
                        nc.tensor.matmul(ps[:], xt_sb[:, mt, j, :], wv_sb[j][:],
                                         start=(j == 0), stop=False)
                    nc.tensor.matmul(ps[:], ones_sb[:, 0:128], bia_sb[0:1, 2 * DIM:3 * DIM],
                                     start=False, stop=True)
                    evict(v_sb[:, mt, :], ps[:], mt % 2 == 0)

            # ================= phase B: attention main loop =================
            with tc.tile_pool(name="pk", bufs=1) as pk:
                for t in range(NT):
                    # per-head scores S_h [n128, m2048] -> bf16
                    sraw = []
                    ei = 0
                    for h in range(8):
                        rc, off = h // 2, (h % 2) * 64
                        sr = pk.tile([128, N], BF16, name="sraw", tag="sraw", bufs=9)
                        sraw.append(sr)
                        for half in range(2):
                            ps = psA.tile([128, 1024], F32, name="pss", tag="pss")
                            for mc in range(2):
                                m0 = 1024 * half + 512 * mc
                                nc.tensor.matmul(
                                    ps[:, 512 * mc:512 * (mc + 1)],
                                    qt_sb[rc][off:off + 64, 128 * t:128 * (t + 1)],
                                    kt_sb[rc][off:off + 64, m0:m0 + 512],
                                    start=True, stop=True)
                            evict(sr[:, 1024 * half:1024 * (half + 1)], ps[:],
                                  ei % 4 == 0)
                            ei += 1
                    # pack into head-interleaved tiles Sint_j [(n16,h), m]
                    sint = [pk.tile([128, N], BF16, name="sint", tag="sint", bufs=3)
                            for _ in range(8)]
                    for h in range(8):
                        for j in range(8):
                            dst = sint[j][:].rearrange("(n h) m -> h n m", h=8)[h]
                            nc.sync.dma_start(dst, sraw[h][NG * j:NG * (j + 1), :])
                    # talking-heads mixes + softmax, per interleaved tile
                    at_h = [pk.tile([128, 8, 8, 128], BF16, name=f"at{i}",
                                    tag="at", bufs=2) for i in range(2)]
                    ei = 0
                    for j in range(8):
                        u = pk.tile([128, N], BF16, name="u", tag="u", bufs=3)
                        zp = pk.tile([128, 2], F32, name="zp", tag="zp", bufs=4)
                        for half in range(2):
                            ps = psA.tile([128, 1024], F32, name="psm1", tag="pss")
                            for mc in range(2):
                                m0 = 1024 * half + 512 * mc
                                nc.tensor.matmul(ps[:, 512 * mc:512 * (mc + 1)],
                                                 t1_sb[:], sint[j][:, m0:m0 + 512],
                                                 start=True, stop=True)
                            nc.scalar.activation(u[:, 1024 * half:1024 * (half + 1)],
                                                 ps[:], AF.Exp,
                                                 accum_out=zp[:, half:half + 1])
                        z = pk.tile([128, 1], F32, name="z", tag="z", bufs=4)
                        nc.vector.tensor_add(z[:], zp[:, 0:1], zp[:, 1:2])
                        rz = pk.tile([128, 1], F32, name="rz", tag="rz", bufs=4)
                        nc.vector.reciprocal(rz[:], z[:])
                        # mix2 weights with softmax division folded in
                        l2 = pk.tile([128, 128], BF16, name="l2", tag="l2", bufs=4)
                        nc.vector.tensor_scalar_mul(l2[:], t2_sb[:], rz[:])
                        a = pk.tile([128, N], BF16, name="a", tag="a", bufs=3)
                        for half in range(2):
                            ps = psA.tile([128, 1024], F32, name="psm2", tag="pss")
                            for mc in range(2):
                                m0 = 1024 * half + 512 * mc
                                nc.tensor.matmul(ps[:, 512 * mc:512 * (mc + 1)],
                                                 l2[:], u[:, m0:m0 + 512],
                                                 start=True, stop=True)
                            evict(a[:, 1024 * half:1024 * (half + 1)], ps[:],
                                  ei % 4 == 0)
                            ei += 1
                        # key-major transpose: at_h[half][p, mc, j, f]
                        #   = A^T at m=(half*8+mc)*128+p, col f=(n16,g)
                        for half in range(2):
                            nc.sync.dma_start_transpose(
                                at_h[half][:, :, j, :],
                                a[:, 1024 * half:1024 * (half + 1)])
                    # attn @ V -> O^T [(g,d), n128]
                    ot = [pk.tile([128, 128], BF16, name=f"ot{i}", tag="ot", bufs=8)
                          for i in range(4)]
                    for g in range(8):
                        ps = psV.tile([64, 128], F32, name="psv", tag="psv")
                        for half in range(2):
                            for mc in range(8):
                                mchunk = half * 8 + mc
                                rhs = at_h[half][:, mc, :, :].rearrange(
                                    "p j (n g) -> p j n g", g=8)[:, :, :, g]
                                nc.tensor.matmul(ps[:],
                                                 v_sb[:, mchunk, 64 * g:64 * (g + 1)],
                                                 rhs, start=(mchunk == 0),
                                                 stop=(mchunk == 15))
                        nc.vector.tensor_copy(
                            ot[g // 2][64 * (g % 2):64 * (g % 2) + 64, :], ps[:])
                    # output projection
                    ps = psB.tile([128, DIM], F32, name="pso", tag="psq")
                    for rc in range(4):
                        nc.tensor.matmul(ps[:], ot[rc][:], wo_sb[rc][:],
                                         start=(rc == 0), stop=False)
                    nc.tensor.matmul(ps[:], ones_sb[:, 0:128], bia_sb[0:1, 3 * DIM:4 * DIM],
                                     start=False, stop=True)
                    yt = pk.tile([128, DIM], F32, name="yt", tag="yt", bufs=3)
                    nc.vector.tensor_copy(yt[:], ps[:])
                    nc.sync.dma_start(y[128 * t:128 * (t + 1), :], yt[:])

    nc.compile()
    return nc


_NC_CACHE = None


def _get_nc():
    global _NC_CACHE
    if _NC_CACHE is None:
        _NC_CACHE = build_nc()
    return _NC_CACHE


def _host_prep(w_qkv, b_qkv, th1, th2, w_out, b_out):
    bf = ml_dtypes.bfloat16
    scale = DH ** -0.5
    w_qkv = np.asarray(w_qkv, dtype=np.float32)
    wq = (w_qkv[:, 0:DIM] * scale).astype(bf)
    wk = w_qkv[:, DIM:2 * DIM].astype(bf)
    wv = w_qkv[:, 2 * DIM:3 * DIM].astype(bf)
    wo = np.asarray(w_out, dtype=np.float32).astype(bf)
    th1 = np.asarray(th1, dtype=np.float32)
    th2 = np.asarray(th2, dtype=np.float32)
    # block-diag templates: T[(n16,h),(n16,g)] = th[g,h]
    t1t = np.zeros((128, 128), dtype=np.float32)
    t2t = np.zeros((128, 128), dtype=np.float32)
    for n16 in range(NG):
        t1t[n16 * 8:n16 * 8 + 8, n16 * 8:n16 * 8 + 8] = th1.T
        t2t[n16 * 8:n16 * 8 + 8, n16 * 8:n16 * 8 + 8] = th2.T
    bqkv = np.asarray(b_qkv, dtype=np.float32)
    bia = np.zeros((1, 4 * DIM), dtype=np.float32)
    bia[0, 0:DIM] = bqkv[0:DIM] * scale     # q bias scaled with w_q
    bia[0, DIM:3 * DIM] = bqkv[DIM:3 * DIM]
    bia[0, 3 * DIM:] = np.asarray(b_out, dtype=np.float32)
    return (wq, wk, wv, wo, t1t.astype(bf), t2t.astype(bf), bia.astype(bf))


def kernel(x, w_qkv, b_qkv, th1, th2, w_out, b_out):
    x = np.asarray(x, dtype=np.float32)
    wq, wk, wv, wo, t1t, t2t, bia = _host_prep(w_qkv, b_qkv, th1, th2, w_out, b_out)
    nc = _get_nc()
    in_maps = []
    for c in range(NCORES):
        b, half = c // 2, c % 2
        in_maps.append({
            "x": np.ascontiguousarray(x[b]),
            "xq": np.ascontiguousarray(x[b, NLOC * half:NLOC * (half + 1), :]),
            "wq": wq, "wk": wk, "wv": wv, "wo": wo,
            "t1t": t1t, "t2t": t2t, "bia": bia,
        })
    res = run_bass_kernel_spmd(nc, in_maps, core_ids=list(range(NCORES)))
    out = np.empty((B, N, DIM), dtype=np.float32)
    for c in range(NCORES):
        b, half = c // 2, c % 2
        out[b, NLOC * half:NLOC * (half + 1), :] = res.results[c]["y"]
    return out


# revision 4
# speedup vs baseline: 1.3003x; 1.3003x over previous
"""Talking-heads attention kernel for Trainium2, 8 NeuronCores.

Problem: B=4, N=2048, DIM=512, H=8, DH=64 talking-heads attention
(qkv proj -> per-head scores -> th1 head-mix -> softmax -> th2 head-mix
 -> attn @ v -> out proj).

Sharding: data-parallel over (batch, query-half): core c handles batch c//2,
query rows [1024*(c%2), 1024*(c%2)+1024).  Communication-free.

Device pipeline per core (all matmuls bf16 with fp32 PSUM accumulation):
  1. x -> bf16 -> xbar-transpose -> x^T; QKV projection on TensorE producing
     Q^T/K^T (head-transposed layout) and V (n-major).
  2. Per 128-query tile: per-head scores S_h = Q_h K_h^T (scale folded into
     w_q on host).
  3. Pack scores into head-interleaved tiles [(n16,h), m] via SBUF->SBUF DMA,
     then both talking-heads mixes run as full-width 128x128 block-diagonal
     matmuls on TensorE.
  4. exp on ScalarE with fused per-row accumulation (softmax denominator Z
     comes for free); the softmax division is folded into the second mix's
     stationary weights (th2[g,h]/Z row scaling), so no elementwise divide
     pass ever touches the [n,m] matrix.
  5. mix2 output is xbar-DMA-transposed to key-major A^T, attn@V runs with
     V as the stationary operand, followed by the output projection.
"""

import sys

sys.path.insert(0, "/opt/trn_rl_repo")

import numpy as np
import ml_dtypes

import concourse.bass as bass
from concourse import bacc
import concourse.mybir as mybir
import concourse.tile as tile
from concourse.bass_utils import run_bass_kernel_spmd

BF16 = mybir.dt.bfloat16
F32 = mybir.dt.float32
AF = mybir.ActivationFunctionType

B, N, DIM = 4, 2048, 512
H, DH = 8, 64
NCORES = 8
NLOC = N // 2          # query rows per core
NT = NLOC // 128       # 8 query tiles per core
MT = N // 128          # 16 key chunks
NG = 16                # n16 group size in interleaved tiles


def build_nc():
    nc = bacc.Bacc()

    x = nc.declare_dram_parameter("x", [N, DIM], F32, isOutput=False)
    xq = nc.declare_dram_parameter("xq", [NLOC, DIM], F32, isOutput=False)
    wq = nc.declare_dram_parameter("wq", [DIM, DIM], BF16, isOutput=False)
    wk = nc.declare_dram_parameter("wk", [DIM, DIM], BF16, isOutput=False)
    wv = nc.declare_dram_parameter("wv", [DIM, DIM], BF16, isOutput=False)
    wo = nc.declare_dram_parameter("wo", [DIM, DIM], BF16, isOutput=False)
    t1t = nc.declare_dram_parameter("t1t", [128, 128], BF16, isOutput=False)
    t2t = nc.declare_dram_parameter("t2t", [128, 128], BF16, isOutput=False)
    # rows: bq (pre-scaled), bk, bv, bo
    bia = nc.declare_dram_parameter("bia", [1, 4 * DIM], BF16, isOutput=False)
    y = nc.declare_dram_parameter("y", [NLOC, DIM], F32, isOutput=True)

    with tile.TileContext(nc) as tc:
        with (
            tc.tile_pool(name="pw", bufs=1) as pw,      # persistent across phases
            tc.tile_pool(name="psA", bufs=2, space="PSUM") as psA,  # [128,1024] mixes
            tc.tile_pool(name="psB", bufs=3, space="PSUM") as psB,  # [128,512] scores/proj
            tc.tile_pool(name="psV", bufs=1, space="PSUM") as psV,  # [64,128] attn@v
        ):
            # persistent small tensors
            wo_sb = [pw.tile([128, DIM], BF16, name=f"wo{i}", tag=f"wo{i}") for i in range(4)]
            for i in range(4):
                nc.sync.dma_start(wo_sb[i][:], wo[128 * i:128 * (i + 1), :])
            t1_sb = pw.tile([128, 128], BF16, name="t1", tag="t1")
            t2_sb = pw.tile([128, 128], BF16, name="t2", tag="t2")
            nc.sync.dma_start(t1_sb[:], t1t[:])
            nc.sync.dma_start(t2_sb[:], t2t[:])
            bia_sb = pw.tile([1, 4 * DIM], BF16, name="bia", tag="bia")
            nc.sync.dma_start(bia_sb[:], bia[:])
            ones_sb = pw.tile([1, DIM], BF16, name="ones", tag="ones")
            nc.any.memset(ones_sb[:], 1.0)

            # persistent activations: Q^T/K^T (head-transposed), V (n-major)
            qt_sb = [pw.tile([128, NLOC], BF16, name=f"qt{i}", tag=f"qt{i}") for i in range(4)]
            kt_sb = [pw.tile([128, N], BF16, name=f"kt{i}", tag=f"kt{i}") for i in range(4)]
            v_sb = pw.tile([128, MT, DIM], BF16, name="v", tag="v")

            def evict(dst, src, use_act):
                if use_act:
                    nc.scalar.copy(dst, src)
                else:
                    nc.vector.tensor_copy(dst, src)

            # ================= phase A: x^T + QKV projection =================
            with tc.tile_pool(name="pxt", bufs=1) as pxt:
                wq_sb = [pxt.tile([128, DIM], BF16, name=f"wq{i}", tag=f"wq{i}") for i in range(4)]
                wk_sb = [pxt.tile([128, DIM], BF16, name=f"wk{i}", tag=f"wk{i}") for i in range(4)]
                wv_sb = [pxt.tile([128, DIM], BF16, name=f"wv{i}", tag=f"wv{i}") for i in range(4)]
                for i in range(4):
                    nc.sync.dma_start(wq_sb[i][:], wq[128 * i:128 * (i + 1), :])
                    nc.sync.dma_start(wk_sb[i][:], wk[128 * i:128 * (i + 1), :])
                    nc.sync.dma_start(wv_sb[i][:], wv[128 * i:128 * (i + 1), :])

                # x -> x^T (bf16): cast-DMA then xbar transpose
                # xt_sb[p, t, j, f]: dim = j*128+p, key row m = t*128+f
                xt_sb = pxt.tile([128, MT, 4, 128], BF16, name="xt", tag="xt")
                for t in range(MT):
                    xb = pxt.tile([128, DIM], BF16, name="xb", tag="xb", bufs=4)
                    nc.gpsimd.dma_start(xb[:], x[128 * t:128 * (t + 1), :])
                    nc.sync.dma_start_transpose(xt_sb[:, t, :, :], xb[:])
                xqt_sb = pxt.tile([128, NT, 4, 128], BF16, name="xqt", tag="xqt")
                for t in range(NT):
                    xb = pxt.tile([128, DIM], BF16, name="xb", tag="xb", bufs=4)
                    nc.gpsimd.dma_start(xb[:], xq[128 * t:128 * (t + 1), :])
                    nc.sync.dma_start_transpose(xqt_sb[:, t, :, :], xb[:])

                ei = 0
                for rc in range(4):            # (h,d) row chunk
                    for nch in range(NLOC // 512):   # Q^T over own query half
                        ps = psB.tile([128, 512], F32, name="psq", tag="pss")
                        for j in range(4):
                            rhs = xqt_sb[:, 4 * nch:4 * (nch + 1), j, :]
                            nc.tensor.matmul(ps[:], wq_sb[j][:, 128 * rc:128 * (rc + 1)],
                                             rhs, start=(j == 0), stop=False)
                        nc.tensor.matmul(ps[:], bia_sb[0:1, 0 * DIM + 128 * rc:0 * DIM + 128 * (rc + 1)],
                                         ones_sb[:, 0:512], start=False, stop=True)
                        evict(qt_sb[rc][:, 512 * nch:512 * (nch + 1)], ps[:], ei % 2 == 0)
                        ei += 1
                    for mch in range(N // 512):      # K^T over all keys
                        ps = psB.tile([128, 512], F32, name="psq", tag="pss")
                        for j in range(4):
                            rhs = xt_sb[:, 4 * mch:4 * (mch + 1), j, :]
                            nc.tensor.matmul(ps[:], wk_sb[j][:, 128 * rc:128 * (rc + 1)],
                                             rhs, start=(j == 0), stop=False)
                        nc.tensor.matmul(ps[:], bia_sb[0:1, 1 * DIM + 128 * rc:1 * DIM + 128 * (rc + 1)],
                                         ones_sb[:, 0:512], start=False, stop=True)
                        evict(kt_sb[rc][:, 512 * mch:512 * (mch + 1)], ps[:], ei % 2 == 0)
                        ei += 1
                for mt in range(MT):           # V n-major
                    ps = psB.tile([128, 512], F32, name="psq", tag="pss")
                    for j in range(4):
                        nc.tensor.matmul(ps[:], xt_sb[:, mt, j, :], wv_sb[j][:],
                                         start=(j == 0), stop=False)
                    nc.tensor.matmul(ps[:], ones_sb[:, 0:128], bia_sb[0:1, 2 * DIM:3 * DIM],
                                     start=False, stop=True)
                    evict(v_sb[:, mt, :], ps[:], mt % 2 == 0)

            # ================= phase B: attention main loop =================
            # Software-pipelined over query tiles: iteration i emits
            # scores(i), mixes(i-1), pack(i), attn@V+outproj(i-2) so the
            # SBUF->SBUF pack and xbar-transpose DMAs always overlap PE work
            # and the PE never idles long enough to re-throttle (HAM).
            with tc.tile_pool(name="pk", bufs=1) as pk:
                st = {}   # per-tile live handles

                def emit_scores(t):
                    sraw = []
                    ei = 0
                    for h in range(8):
                        rc, off = h // 2, (h % 2) * 64
                        sr = pk.tile([128, N], BF16, name="sraw", tag="sraw", bufs=9)
                        sraw.append(sr)
                        for mc in range(4):
                            ps = psB.tile([128, 512], F32, name="pss", tag="pss")
                            nc.tensor.matmul(
                                ps[:],
                                qt_sb[rc][off:off + 64, 128 * t:128 * (t + 1)],
                                kt_sb[rc][off:off + 64, 512 * mc:512 * (mc + 1)],
                                start=True, stop=True)
                            evict(sr[:, 512 * mc:512 * (mc + 1)], ps[:], ei % 4 == 0)
                            ei += 1
                    st[t] = {"sraw": sraw}

                def emit_pack(t):
                    sraw = st[t]["sraw"]
                    sint = [pk.tile([128, N], BF16, name="sint", tag="sint", bufs=11)
                            for _ in range(8)]
                    for h in range(8):
                        for j in range(8):
                            dst = sint[j][:].rearrange("(n h) m -> h n m", h=8)[h]
                            nc.sync.dma_start(dst, sraw[h][NG * j:NG * (j + 1), :])
                    st[t]["sint"] = sint

                def emit_mixes(t):
                    sint = st[t]["sint"]
                    at_h = [pk.tile([128, 8, 8, 128], BF16, name=f"at{i}",
                                    tag="at", bufs=2) for i in range(2)]
                    ei = 0
                    for j in range(8):
                        u = pk.tile([128, N], BF16, name="u", tag="u", bufs=2)
                        zp = pk.tile([128, 2], F32, name="zp", tag="zp", bufs=4)
                        for half in range(2):
                            ps = psA.tile([128, 1024], F32, name="psm1", tag="psm")
                            for mc in range(2):
                                m0 = 1024 * half + 512 * mc
                                nc.tensor.matmul(ps[:, 512 * mc:512 * (mc + 1)],
                                                 t1_sb[:], sint[j][:, m0:m0 + 512],
                                                 start=True, stop=True)
                            nc.scalar.activation(u[:, 1024 * half:1024 * (half + 1)],
                                                 ps[:], AF.Exp,
                                                 accum_out=zp[:, half:half + 1])
                        z = pk.tile([128, 1], F32, name="z", tag="z", bufs=4)
                        nc.vector.tensor_add(z[:], zp[:, 0:1], zp[:, 1:2])
                        rz = pk.tile([128, 1], F32, name="rz", tag="rz", bufs=4)
                        nc.vector.reciprocal(rz[:], z[:])
                        l2 = pk.tile([128, 128], BF16, name="l2", tag="l2", bufs=4)
                        nc.vector.tensor_scalar_mul(l2[:], t2_sb[:], rz[:])
                        a = pk.tile([128, N], BF16, name="a", tag="a", bufs=2)
                        for half in range(2):
                            ps = psA.tile([128, 1024], F32, name="psm2", tag="psm")
                            for mc in range(2):
                                m0 = 1024 * half + 512 * mc
                                nc.tensor.matmul(ps[:, 512 * mc:512 * (mc + 1)],
                                                 l2[:], u[:, m0:m0 + 512],
                                                 start=True, stop=True)
                            evict(a[:, 1024 * half:1024 * (half + 1)], ps[:],
                                  ei % 2 == 0)
                            ei += 1
                        for half in range(2):
                            nc.sync.dma_start_transpose(
                                at_h[half][:, :, j, :],
                                a[:, 1024 * half:1024 * (half + 1)])
                    st[t]["at"] = at_h

                def emit_av(t):
                    at_h = st[t]["at"]
                    ot = [pk.tile([128, 128], BF16, name=f"ot{i}", tag="ot", bufs=8)
                          for i in range(4)]
                    for g in range(8):
                        ps = psV.tile([64, 128], F32, name="psv", tag="psv")
                        for half in range(2):
                            for mc in range(8):
                                mchunk = half * 8 + mc
                                rhs = at_h[half][:, mc, :, :].rearrange(
                                    "p j (n g) -> p j n g", g=8)[:, :, :, g]
                                nc.tensor.matmul(ps[:],
                                                 v_sb[:, mchunk, 64 * g:64 * (g + 1)],
                                                 rhs, start=(mchunk == 0),
                                                 stop=(mchunk == 15))
                        nc.vector.tensor_copy(
                            ot[g // 2][64 * (g % 2):64 * (g % 2) + 64, :], ps[:])
                    ps = psB.tile([128, DIM], F32, name="pso", tag="pss")
                    for rc in range(4):
                        nc.tensor.matmul(ps[:], ot[rc][:], wo_sb[rc][:],
                                         start=(rc == 0), stop=False)
                    nc.tensor.matmul(ps[:], ones_sb[:, 0:128], bia_sb[0:1, 3 * DIM:4 * DIM],
                                     start=False, stop=True)
                    yt = pk.tile([128, DIM], F32, name="yt", tag="yt", bufs=3)
                    nc.vector.tensor_copy(yt[:], ps[:])
                    nc.sync.dma_start(y[128 * t:128 * (t + 1), :], yt[:])
                    del st[t]

                for i in range(NT + 2):
                    if i < NT:
                        emit_scores(i)
                    if 1 <= i <= NT:
                        emit_mixes(i - 1)
                    if i < NT:
                        emit_pack(i)
                    if i >= 2:
                        emit_av(i - 2)

    nc.compile()
    return nc


_NC_CACHE = None


def _get_nc():
    global _NC_CACHE
    if _NC_CACHE is None:
        _NC_CACHE = build_nc()
    return _NC_CACHE


def _host_prep(w_qkv, b_qkv, th1, th2, w_out, b_out):
    bf = ml_dtypes.bfloat16
    scale = DH ** -0.5
    w_qkv = np.asarray(w_qkv, dtype=np.float32)
    wq = (w_qkv[:, 0:DIM] * scale).astype(bf)
    wk = w_qkv[:, DIM:2 * DIM].astype(bf)
    wv = w_qkv[:, 2 * DIM:3 * DIM].astype(bf)
    wo = np.asarray(w_out, dtype=np.float32).astype(bf)
    th1 = np.asarray(th1, dtype=np.float32)
    th2 = np.asarray(th2, dtype=np.float32)
    # block-diag templates: T[(n16,h),(n16,g)] = th[g,h]
    t1t = np.zeros((128, 128), dtype=np.float32)
    t2t = np.zeros((128, 128), dtype=np.float32)
    for n16 in range(NG):
        t1t[n16 * 8:n16 * 8 + 8, n16 * 8:n16 * 8 + 8] = th1.T
        t2t[n16 * 8:n16 * 8 + 8, n16 * 8:n16 * 8 + 8] = th2.T
    bqkv = np.asarray(b_qkv, dtype=np.float32)
    bia = np.zeros((1, 4 * DIM), dtype=np.float32)
    bia[0, 0:DIM] = bqkv[0:DIM] * scale     # q bias scaled with w_q
    bia[0, DIM:3 * DIM] = bqkv[DIM:3 * DIM]
    bia[0, 3 * DIM:] = np.asarray(b_out, dtype=np.float32)
    return (wq, wk, wv, wo, t1t.astype(bf), t2t.astype(bf), bia.astype(bf))


def kernel(x, w_qkv, b_qkv, th1, th2, w_out, b_out):
    x = np.asarray(x, dtype=np.float32)
    wq, wk, wv, wo, t1t, t2t, bia = _host_prep(w_qkv, b_qkv, th1, th2, w_out, b_out)
    nc = _get_nc()
    in_maps = []
    for c in range(NCORES):
        b, half = c // 2, c % 2
        in_maps.append({
            "x": np.ascontiguousarray(x[b]),
            "xq": np.ascontiguousarray(x[b, NLOC * half:NLOC * (half + 1), :]),
            "wq": wq, "wk": wk, "wv": wv, "wo": wo,
            "t1t": t1t, "t2t": t2t, "bia": bia,
        })
    res = run_bass_kernel_spmd(nc, in_maps, core_ids=list(range(NCORES)))
    out = np.empty((B, N, DIM), dtype=np.float32)
    for c in range(NCORES):
        b, half = c // 2, c % 2
        out[b, NLOC * half:NLOC * (half + 1), :] = res.results[c]["y"]
    return out


# revision 8
# speedup vs baseline: 1.3114x; 1.0086x over previous
"""Talking-heads attention kernel for Trainium2, 8 NeuronCores.

Problem: B=4, N=2048, DIM=512, H=8, DH=64 talking-heads attention
(qkv proj -> per-head scores -> th1 head-mix -> softmax -> th2 head-mix
 -> attn @ v -> out proj).

Sharding: data-parallel over (batch, query-half): core c handles batch c//2,
query rows [1024*(c%2), 1024*(c%2)+1024).  Communication-free.

Device pipeline per core (all matmuls bf16 with fp32 PSUM accumulation):
  1. x -> bf16 -> xbar-transpose -> x^T; QKV projection on TensorE producing
     Q^T/K^T (head-transposed layout) and V (n-major).
  2. Per 128-query tile: per-head scores S_h = Q_h K_h^T (scale folded into
     w_q on host).
  3. Pack scores into head-interleaved tiles [(n16,h), m] via SBUF->SBUF DMA,
     then both talking-heads mixes run as full-width 128x128 block-diagonal
     matmuls on TensorE.
  4. exp on ScalarE with fused per-row accumulation (softmax denominator Z
     comes for free); the softmax division is folded into the second mix's
     stationary weights (th2[g,h]/Z row scaling), so no elementwise divide
     pass ever touches the [n,m] matrix.
  5. mix2 output is xbar-DMA-transposed to key-major A^T, attn@V runs with
     V as the stationary operand, followed by the output projection.
"""

import sys

sys.path.insert(0, "/opt/trn_rl_repo")

import numpy as np
import ml_dtypes

import concourse.bass as bass
from concourse import bacc
import concourse.mybir as mybir
import concourse.tile as tile
from concourse.bass_utils import run_bass_kernel_spmd

BF16 = mybir.dt.bfloat16
F32 = mybir.dt.float32
AF = mybir.ActivationFunctionType

B, N, DIM = 4, 2048, 512
H, DH = 8, 64
NCORES = 8
NLOC = N // 2          # query rows per core
NT = NLOC // 128       # 8 query tiles per core
MT = N // 128          # 16 key chunks
NG = 16                # n16 group size in interleaved tiles


def build_nc():
    nc = bacc.Bacc()

    x = nc.declare_dram_parameter("x", [N, DIM], F32, isOutput=False)
    xq = nc.declare_dram_parameter("xq", [NLOC, DIM], F32, isOutput=False)
    wq = nc.declare_dram_parameter("wq", [DIM, DIM], BF16, isOutput=False)
    wk = nc.declare_dram_parameter("wk", [DIM, DIM], BF16, isOutput=False)
    wv = nc.declare_dram_parameter("wv", [DIM, DIM], BF16, isOutput=False)
    wo = nc.declare_dram_parameter("wo", [DIM, DIM], BF16, isOutput=False)
    t1t = nc.declare_dram_parameter("t1t", [128, 128], BF16, isOutput=False)
    t2t = nc.declare_dram_parameter("t2t", [128, 128], BF16, isOutput=False)
    # rows: bq (pre-scaled), bk, bv, bo
    bia = nc.declare_dram_parameter("bia", [1, 4 * DIM], BF16, isOutput=False)
    y = nc.declare_dram_parameter("y", [NLOC, DIM], F32, isOutput=True)

    with tile.TileContext(nc) as tc:
        with (
            tc.tile_pool(name="pw", bufs=1) as pw,      # persistent across phases
            tc.tile_pool(name="psA", bufs=2, space="PSUM") as psA,  # [128,1024] mixes
            tc.tile_pool(name="psB", bufs=3, space="PSUM") as psB,  # [128,512] scores/proj
            tc.tile_pool(name="psV", bufs=1, space="PSUM") as psV,  # [128,128] attn@v
        ):
            # persistent small tensors
            wo_sb = [pw.tile([128, DIM], BF16, name=f"wo{i}", tag=f"wo{i}") for i in range(4)]
            for i in range(4):
                nc.sync.dma_start(wo_sb[i][:], wo[128 * i:128 * (i + 1), :])
            t1_sb = pw.tile([128, 128], BF16, name="t1", tag="t1")
            t2_sb = pw.tile([128, 128], BF16, name="t2", tag="t2")
            nc.sync.dma_start(t1_sb[:], t1t[:])
            nc.sync.dma_start(t2_sb[:], t2t[:])
            bia_sb = pw.tile([1, 4 * DIM], BF16, name="bia", tag="bia")
            nc.sync.dma_start(bia_sb[:], bia[:])
            ones_sb = pw.tile([1, DIM], BF16, name="ones", tag="ones")
            nc.any.memset(ones_sb[:], 1.0)

            # persistent activations: Q^T/K^T (head-transposed), V (n-major)
            qt_sb = [pw.tile([128, NLOC], BF16, name=f"qt{i}", tag=f"qt{i}") for i in range(4)]
            kt_sb = [pw.tile([128, N], BF16, name=f"kt{i}", tag=f"kt{i}") for i in range(4)]
            v_sb = pw.tile([128, MT, DIM], BF16, name="v", tag="v")

            def evict(dst, src, use_act):
                if use_act:
                    nc.scalar.copy(dst, src)
                else:
                    nc.vector.tensor_copy(dst, src)

            # ================= phase A: x^T + QKV projection =================
            with tc.tile_pool(name="pxt", bufs=1) as pxt:
                wq_sb = [pxt.tile([128, DIM], BF16, name=f"wq{i}", tag=f"wq{i}") for i in range(4)]
                wk_sb = [pxt.tile([128, DIM], BF16, name=f"wk{i}", tag=f"wk{i}") for i in range(4)]
                wv_sb = [pxt.tile([128, DIM], BF16, name=f"wv{i}", tag=f"wv{i}") for i in range(4)]
                for i in range(4):
                    nc.sync.dma_start(wq_sb[i][:], wq[128 * i:128 * (i + 1), :])
                    nc.sync.dma_start(wk_sb[i][:], wk[128 * i:128 * (i + 1), :])
                    nc.sync.dma_start(wv_sb[i][:], wv[128 * i:128 * (i + 1), :])

                # x -> x^T (bf16): cast-DMA then xbar transpose
                # xt_sb[p, t, j, f]: dim = j*128+p, key row m = t*128+f
                xt_sb = pxt.tile([128, MT, 4, 128], BF16, name="xt", tag="xt")
                for t in range(MT):
                    xb = pxt.tile([128, DIM], BF16, name="xb", tag="xb", bufs=4)
                    nc.gpsimd.dma_start(xb[:], x[128 * t:128 * (t + 1), :])
                    nc.sync.dma_start_transpose(xt_sb[:, t, :, :], xb[:])
                xqt_sb = pxt.tile([128, NT, 4, 128], BF16, name="xqt", tag="xqt")
                for t in range(NT):
                    xb = pxt.tile([128, DIM], BF16, name="xb", tag="xb", bufs=4)
                    nc.gpsimd.dma_start(xb[:], xq[128 * t:128 * (t + 1), :])
                    nc.sync.dma_start_transpose(xqt_sb[:, t, :, :], xb[:])

                ei = 0
                for rc in range(4):            # (h,d) row chunk
                    for nch in range(NLOC // 512):   # Q^T over own query half
                        ps = psB.tile([128, 512], F32, name="psq", tag="pss")
                        for j in range(4):
                            rhs = xqt_sb[:, 4 * nch:4 * (nch + 1), j, :]
                            nc.tensor.matmul(ps[:], wq_sb[j][:, 128 * rc:128 * (rc + 1)],
                                             rhs, start=(j == 0), stop=False)
                        nc.tensor.matmul(ps[:], bia_sb[0:1, 0 * DIM + 128 * rc:0 * DIM + 128 * (rc + 1)],
                                         ones_sb[:, 0:512], start=False, stop=True)
                        evict(qt_sb[rc][:, 512 * nch:512 * (nch + 1)], ps[:], ei % 2 == 0)
                        ei += 1
                    for mch in range(N // 512):      # K^T over all keys
                        ps = psB.tile([128, 512], F32, name="psq", tag="pss")
                        for j in range(4):
                            rhs = xt_sb[:, 4 * mch:4 * (mch + 1), j, :]
                            nc.tensor.matmul(ps[:], wk_sb[j][:, 128 * rc:128 * (rc + 1)],
                                             rhs, start=(j == 0), stop=False)
                        nc.tensor.matmul(ps[:], bia_sb[0:1, 1 * DIM + 128 * rc:1 * DIM + 128 * (rc + 1)],
                                         ones_sb[:, 0:512], start=False, stop=True)
                        evict(kt_sb[rc][:, 512 * mch:512 * (mch + 1)], ps[:], ei % 2 == 0)
                        ei += 1
                for mt in range(MT):           # V n-major
                    ps = psB.tile([128, 512], F32, name="psq", tag="pss")
                    for j in range(4):
                        nc.tensor.matmul(ps[:], xt_sb[:, mt, j, :], wv_sb[j][:],
                                         start=(j == 0), stop=False)
                    nc.tensor.matmul(ps[:], ones_sb[:, 0:128], bia_sb[0:1, 2 * DIM:3 * DIM],
                                     start=False, stop=True)
                    evict(v_sb[:, mt, :], ps[:], mt % 2 == 0)

            # ================= phase B: attention main loop =================
            # Software-pipelined over query tiles: iteration i emits
            # scores(i), mixes(i-1), pack(i), attn@V+outproj(i-2) so the
            # SBUF->SBUF pack and xbar-transpose DMAs always overlap PE work
            # and the PE never idles long enough to re-throttle (HAM).
            with tc.tile_pool(name="pk", bufs=1) as pk:
                st = {}   # per-tile live handles

                def emit_scores(t):
                    sraw = []
                    ei = 0
                    for h in range(8):
                        rc, off = h // 2, (h % 2) * 64
                        sr = pk.tile([128, N], BF16, name="sraw", tag="sraw", bufs=8)
                        sraw.append(sr)
                        for mc in range(4):
                            ps = psB.tile([128, 512], F32, name="pss", tag="pss")
                            nc.tensor.matmul(
                                ps[:],
                                qt_sb[rc][off:off + 64, 128 * t:128 * (t + 1)],
                                kt_sb[rc][off:off + 64, 512 * mc:512 * (mc + 1)],
                                start=True, stop=True)
                            evict(sr[:, 512 * mc:512 * (mc + 1)], ps[:], ei % 4 == 0)
                            ei += 1
                    st[t] = {"sraw": sraw}

                def emit_pack(t):
                    sraw = st[t]["sraw"]
                    sint = pk.tile([128, 8, N], BF16, name="sint", tag="sint", bufs=2)
                    # sint[(n16,h), j, m] <- sraw_h[16j+n16, m]
                    for h in range(8):
                        for j in range(8):
                            dst = sint[:, j, :].rearrange("(n h) m -> h n m", h=8)[h]
                            nc.sync.dma_start(dst, sraw[h][NG * j:NG * (j + 1), :])
                    st[t]["sint"] = sint

                def emit_mixes(t):
                    sint = st[t]["sint"]
                    # at_h[half][p, j, mc, f]: A^T row m = (half*8+mc)*128+p,
                    # col f = (n16,g); per-j xbar dst [:, j, :, :] is contiguous
                    at_h = [pk.tile([128, 8, 8, 128], BF16, name=f"at{i}",
                                    tag="at", bufs=2) for i in range(2)]
                    ei = 0
                    for j in range(8):
                        u = pk.tile([128, N], BF16, name="u", tag="u", bufs=2)
                        # cols: zp0, zp1, z, rz
                        zz = pk.tile([128, 4], F32, name="zz", tag="zz", bufs=4)
                        for half in range(2):
                            ps = psA.tile([128, 1024], F32, name="psm1", tag="psm")
                            for mc in range(2):
                                m0 = 1024 * half + 512 * mc
                                nc.tensor.matmul(ps[:, 512 * mc:512 * (mc + 1)],
                                                 t1_sb[:], sint[:, j, m0:m0 + 512],
                                                 start=True, stop=True)
                            nc.scalar.activation(u[:, 1024 * half:1024 * (half + 1)],
                                                 ps[:], AF.Exp,
                                                 accum_out=zz[:, half:half + 1])
                        nc.vector.tensor_add(zz[:, 2:3], zz[:, 0:1], zz[:, 1:2])
                        nc.vector.reciprocal(zz[:, 3:4], zz[:, 2:3])
                        l2 = pk.tile([128, 128], BF16, name="l2", tag="l2", bufs=2)
                        nc.vector.tensor_scalar_mul(l2[:], t2_sb[:], zz[:, 3:4])
                        a = pk.tile([128, N], BF16, name="a", tag="a", bufs=2)
                        for half in range(2):
                            ps = psA.tile([128, 1024], F32, name="psm2", tag="psm")
                            for mc in range(2):
                                m0 = 1024 * half + 512 * mc
                                nc.tensor.matmul(ps[:, 512 * mc:512 * (mc + 1)],
                                                 l2[:], u[:, m0:m0 + 512],
                                                 start=True, stop=True)
                            evict(a[:, 1024 * half:1024 * (half + 1)], ps[:],
                                  ei % 2 == 0)
                            ei += 1
                        for half in range(2):
                            nc.sync.dma_start_transpose(
                                at_h[half][:, j, :, :],
                                a[:, 1024 * half:1024 * (half + 1)])
                    st[t]["at"] = at_h

                def emit_av(t):
                    at_h = st[t]["at"]
                    otb = pk.tile([128, 4, 128], BF16, name="otb", tag="otb", bufs=2)
                    for rc in range(4):
                        ps = psV.tile([128, 128], F32, name="psv", tag="psv")
                        for gi in range(2):
                            g = 2 * rc + gi
                            for half in range(2):
                                for mc in range(8):
                                    mchunk = half * 8 + mc
                                    rhs = at_h[half][:, :, mc, :].rearrange(
                                        "p j (n g) -> p j n g", g=8)[:, :, :, g]
                                    nc.tensor.matmul(
                                        ps[64 * gi:64 * (gi + 1), :],
                                        v_sb[:, mchunk, 64 * g:64 * (g + 1)],
                                        rhs, start=(mchunk == 0),
                                        stop=(mchunk == 15))
                        nc.vector.tensor_copy(otb[:, rc, :], ps[:])
                    ps = psB.tile([128, DIM], F32, name="pso", tag="pss")
                    for rc in range(4):
                        nc.tensor.matmul(ps[:], otb[:, rc, :], wo_sb[rc][:],
                                         start=(rc == 0), stop=False)
                    nc.tensor.matmul(ps[:], ones_sb[:, 0:128], bia_sb[0:1, 3 * DIM:4 * DIM],
                                     start=False, stop=True)
                    yt = pk.tile([128, DIM], F32, name="yt", tag="yt", bufs=2)
                    nc.vector.tensor_copy(yt[:], ps[:])
                    nc.sync.dma_start(y[128 * t:128 * (t + 1), :], yt[:])
                    del st[t]

                for i in range(NT + 2):
                    if i < NT:
                        emit_scores(i)
                    if 1 <= i <= NT:
                        emit_mixes(i - 1)
                    if i < NT:
                        emit_pack(i)
                    if i >= 2:
                        emit_av(i - 2)

    nc.compile()
    return nc


_NC_CACHE = None


def _get_nc():
    global _NC_CACHE
    if _NC_CACHE is None:
        _NC_CACHE = build_nc()
    return _NC_CACHE


def _host_prep(w_qkv, b_qkv, th1, th2, w_out, b_out):
    bf = ml_dtypes.bfloat16
    scale = DH ** -0.5
    w_qkv = np.asarray(w_qkv, dtype=np.float32)
    wq = (w_qkv[:, 0:DIM] * scale).astype(bf)
    wk = w_qkv[:, DIM:2 * DIM].astype(bf)
    wv = w_qkv[:, 2 * DIM:3 * DIM].astype(bf)
    wo = np.asarray(w_out, dtype=np.float32).astype(bf)
    th1 = np.asarray(th1, dtype=np.float32)
    th2 = np.asarray(th2, dtype=np.float32)
    # block-diag templates: T[(n16,h),(n16,g)] = th[g,h]
    t1t = np.zeros((128, 128), dtype=np.float32)
    t2t = np.zeros((128, 128), dtype=np.float32)
    for n16 in range(NG):
        t1t[n16 * 8:n16 * 8 + 8, n16 * 8:n16 * 8 + 8] = th1.T
        t2t[n16 * 8:n16 * 8 + 8, n16 * 8:n16 * 8 + 8] = th2.T
    bqkv = np.asarray(b_qkv, dtype=np.float32)
    bia = np.zeros((1, 4 * DIM), dtype=np.float32)
    bia[0, 0:DIM] = bqkv[0:DIM] * scale     # q bias scaled with w_q
    bia[0, DIM:3 * DIM] = bqkv[DIM:3 * DIM]
    bia[0, 3 * DIM:] = np.asarray(b_out, dtype=np.float32)
    return (wq, wk, wv, wo, t1t.astype(bf), t2t.astype(bf), bia.astype(bf))


def kernel(x, w_qkv, b_qkv, th1, th2, w_out, b_out):
    x = np.asarray(x, dtype=np.float32)
    wq, wk, wv, wo, t1t, t2t, bia = _host_prep(w_qkv, b_qkv, th1, th2, w_out, b_out)
    nc = _get_nc()
    in_maps = []
    for c in range(NCORES):
        b, half = c // 2, c % 2
        in_maps.append({
            "x": np.ascontiguousarray(x[b]),
            "xq": np.ascontiguousarray(x[b, NLOC * half:NLOC * (half + 1), :]),
            "wq": wq, "wk": wk, "wv": wv, "wo": wo,
            "t1t": t1t, "t2t": t2t, "bia": bia,
        })
    res = run_bass_kernel_spmd(nc, in_maps, core_ids=list(range(NCORES)))
    out = np.empty((B, N, DIM), dtype=np.float32)
    for c in range(NCORES):
        b, half = c // 2, c % 2
        out[b, NLOC * half:NLOC * (half + 1), :] = res.results[c]["y"]
    return out
